# revision 48
# baseline (speedup 1.0000x reference)
# Trainium2 Bass kernel for nn_CrossAttention (B=1, I=J=1024, C_S=1024,
# C_Z=128, H=16, D=64), sharded over the query dim i across 8 NeuronCores.
#
# Per-core program (i-slice of 128 query rows):
#   qT = (Wq s_c^T + bq)/sqrt(D)  kT = Wk k_in^T   v = k_in Wv^T  (bf16 matmuls)
#   z[i,j,h] = sum_c bias[i,j,c] Wz[c,h]   via per-j matmuls with Wz
#   qk[i,j] = qT_h^T kT_h  (PSUM), z added in-place in PSUM, exp on ACT
#              (softmax over j without max-subtraction; logits are O(1))
#   o[i,:] = sum_j exp^T v_aug[j]  with mask[j] in an extra v column so the
#              denominator comes out of the same matmul chain
#   out = (sigmoid(s_c Wg^T) * o) @ Wo^T
#
# All bulk inputs are pre-cast to bf16 AND pre-transposed on the host into
# the layouts the tensor engine contracts over ([contraction, free]):
# weights as W^T, k_in as k_in^T, s as s^T, and bias as [c, j, i] per core.
# This removes ~1400 on-chip PE transposes and all their PSUM->SBUF
# evacuation copies; the z-path is just DMA -> per-j matmuls.  The z stream
# is interleaved at fine granularity with the projection matmuls and 3
# attention passes (512/256/256 j).
#
# kernel(**inputs) takes FULL inputs, shards on host, runs SPMD on cores 0-7,
# gathers to the full [1, 1024, 1024] output.

import numpy as np

B, I, J, CS, CZ, H, D = 1, 1024, 1024, 1024, 128, 16, 64
NCORES = 8
NI = I // NCORES  # 128 query rows per core
P = 128
NCHUNK = 16  # bias chunks of 64 j
CJ = J // NCHUNK  # 64 j per chunk
NUNIT = J // 8  # z work units of 8 j

_last_results = None


def _build_program():
    from contextlib import ExitStack

    import concourse.mybir as mybir
    import concourse.tile as tile
    from concourse import bacc
    from concourse.masks import make_identity

    f32 = mybir.dt.float32
    bf16 = mybir.dt.bfloat16
    AF = mybir.ActivationFunctionType
    ALU = mybir.AluOpType

    nc = bacc.Bacc("TRN2", target_bir_lowering=False, debug=False)

    # ---- dram io (host-pretransposed, bf16) ----
    sT_d = nc.dram_tensor("sT", [CS, NI], bf16, kind="ExternalInput").ap()
    biasT_d = nc.dram_tensor("biasT", [CZ, J, NI], bf16, kind="ExternalInput").ap()
    kinT_d = nc.dram_tensor("kinT", [CS, J], bf16, kind="ExternalInput").ap()
    mask = nc.dram_tensor("mask", [J], f32, kind="ExternalInput").ap()
    wqT_d = nc.dram_tensor("wqT", [CS, CS], bf16, kind="ExternalInput").ap()
    wkT_d = nc.dram_tensor("wkT", [CS, CS], bf16, kind="ExternalInput").ap()
    wvT_d = nc.dram_tensor("wvT", [CS, CS], bf16, kind="ExternalInput").ap()
    wgT_d = nc.dram_tensor("wgT", [CS, CS], bf16, kind="ExternalInput").ap()
    woT_d = nc.dram_tensor("woT", [CS, CS], bf16, kind="ExternalInput").ap()
    b_q = nc.dram_tensor("b_q", [CS], f32, kind="ExternalInput").ap()
    w_z = nc.dram_tensor("w_z", [CZ, H], bf16, kind="ExternalInput").ap()
    out_d = nc.dram_tensor("out", [NI, CS], f32, kind="ExternalOutput").ap()

    KC = CS // P  # 8 contraction chunks

    with tile.TileContext(nc) as tc, ExitStack() as ctx:
        pool = lambda name, bufs: ctx.enter_context(tc.tile_pool(name=name, bufs=bufs))
        ppool = lambda name, bufs: ctx.enter_context(
            tc.tile_pool(name=name, bufs=bufs, space="PSUM")
        )

        const = pool("const", 1)
        wt_p = pool("wt", 2)
        kin_p = pool("kin", 1)
        small_p = pool("small", 1)
        big_p = pool("big", 1)
        bstage_p = pool("bstage", 3)
        et_p = pool("et", 2)
        r_p = pool("r", 2)
        outs_p = pool("outs", 1)

        tpsum = ppool("tpsum", 2)  # transpose targets (exp / gated-out)
        zps = ppool("zps", 1)  # z accumulation [128,512] f32
        bigps = ppool("bigps", 2)  # projection accumulators
        qkps = ppool("qkps", 2)  # attention logits f32
        ops = ppool("ops", 1)  # attention output f32

        ident = const.tile([P, P], bf16)
        make_identity(nc, ident)
        wz_s = const.tile([CZ, H], bf16)
        nc.sync.dma_start(wz_s, w_z)
        bq_s = const.tile([P, KC], f32)
        nc.sync.dma_start(bq_s, b_q.rearrange("(fo p) -> p fo", p=P))
        mask_s = const.tile([P, KC], f32)
        nc.sync.dma_start(mask_s, mask.rearrange("(jo p) -> p jo", p=P))

        # ---- z state ----
        # z_s layout: [i_part, block(32), j_local(32), h(16)] bf16 —
        # blocks of 32 j, independent of the 64-j DMA chunking
        z_s = big_p.tile([P, 32, 32, H], bf16, tag="z")
        bstage = [None] * NCHUNK

        def emit_chunk_dma(c):
            # biasT chunk: [c_part(128), j(32), i(128)] — 8 KB contiguous
            # per partition line
            bt = bstage_p.tile([P, CJ, NI], bf16, tag="bt", name=f"bt_{c}")
            bstage[c] = bt
            if c == 0:
                # split the first chunk so the PE can start early
                nc.sync.dma_start(bt[:, :8, :], biasT_d[:, :8, :])
                nc.sync.dma_start(bt[:, 8:, :], biasT_d[:, 8:CJ, :])
            else:
                nc.sync.dma_start(bt, biasT_d[:, c * CJ : (c + 1) * CJ, :])

        zstate = {"u": 0, "zp": None}

        def emit_z_unit():
            # one unit = 8 j: 8 z-matmuls straight off the DMA'd biasT
            u = zstate["u"]
            assert u < NUNIT
            zstate["u"] = u + 1
            c, uc = divmod(u, 8)  # 64-j chunk, unit-in-chunk
            b, ub = divmod(u, 4)  # 32-j z_s block, unit-in-block
            if uc == 0:
                if bstage[c] is None:
                    emit_chunk_dma(c)
                for pf in (c + 1, c + 2):
                    if pf < NCHUNK and bstage[pf] is None:
                        emit_chunk_dma(pf)
            if ub == 0:
                zstate["zp"] = zps.tile([P, 512], f32, tag="zp", name=f"zp_{u}")
            bt = bstage[c]
            zp = zstate["zp"]
            for jl in range(8):
                jj = uc * 8 + jl
                nc.tensor.matmul(
                    zp[:, (ub * 8 + jl) * H : (ub * 8 + jl + 1) * H],
                    bt[:, jj, :],
                    wz_s,
                    start=True,
                    stop=True,
                )
            if ub == 3:
                nc.scalar.copy(z_s[:, b, :, :].rearrange("p a b -> p (a b)"), zp)

        def zsteps(n):
            for _ in range(min(n, NUNIT - zstate["u"])):
                emit_z_unit()

        def z_barrier(n):
            while zstate["u"] < n:
                emit_z_unit()

        # ---- kick off: first bias chunk and the small transposed loads;
        # wq before kinT so the Q projection is never weight-starved ----
        emit_chunk_dma(0)
        sT = small_p.tile([P, KC, NI], bf16, tag="sT")
        nc.sync.dma_start(sT, sT_d.rearrange("(co p) i -> p co i", p=P))

        def load_wT(w_ap, name):
            wT = wt_p.tile([P, KC, CS], bf16, tag="wt", name=name)
            wr = w_ap.rearrange("(co p) f -> p co f", p=P)
            nc.sync.dma_start(wT[:, :4, :], wr[:, :4, :])
            nc.sync.dma_start(wT[:, 4:, :], wr[:, 4:, :])
            return wT

        wqT = load_wT(wqT_d, "wqT")
        zsteps(2)

        kinT = kin_p.tile([P, KC, J], bf16, tag="kinT")
        kr = kinT_d.rearrange("(co p) j -> p co j", p=P)
        nc.sync.dma_start(kinT[:, :4, :], kr[:, :4, :])
        nc.sync.dma_start(kinT[:, 4:, :], kr[:, 4:, :])

        # ---- Wq + Q projection: qT[f,i] = (Wq s^T + bq)/sqrt(D) ----
        qT = small_p.tile([P, KC, NI], bf16, tag="qT")
        for fo in range(KC):
            ps = bigps.tile([P, 512], f32, tag="big", name=f"qp_{fo}")
            for co in range(KC):
                nc.tensor.matmul(
                    ps[:, :NI],
                    wqT[:, co, fo * P : (fo + 1) * P],
                    sT[:, co, :],
                    start=(co == 0),
                    stop=(co == KC - 1),
                )
            nc.vector.tensor_scalar(
                qT[:, fo, :],
                ps[:, :NI],
                bq_s[:, fo : fo + 1],
                1.0 / np.sqrt(D),
                ALU.add,
                ALU.mult,
            )
            zsteps(1)

        # ---- Wk + K projection: kT[f,j] = Wk k_in^T ----
        wkT = load_wT(wkT_d, "wkT")
        kT = big_p.tile([P, KC, J], bf16, tag="kT")

        engflip = [0]

        def copy_alt(out, in_):
            engflip[0] = (engflip[0] + 1) % 3
            if engflip[0] == 0:
                nc.scalar.copy(out, in_)
            else:
                nc.vector.tensor_copy(out, in_)

        def emit_k_proj(fo, jh):
            ps = bigps.tile([P, 512], f32, tag="big", name=f"kp_{fo}_{jh}")
            for co in range(KC):
                nc.tensor.matmul(
                    ps,
                    wkT[:, co, fo * P : (fo + 1) * P],
                    kinT[:, co, jh * 512 : (jh + 1) * 512],
                    start=(co == 0),
                    stop=(co == KC - 1),
                )
            copy_alt(kT[:, fo, jh * 512 : (jh + 1) * 512], ps)

        for fo in range(KC):  # j half 0 first: unblocks attn pass 0
            emit_k_proj(fo, 0)
            zsteps(2)

        # ---- Wv + V projection: v[j, h, d|mask] ----
        wvT = load_wT(wvT_d, "wvT")
        v_s = big_p.tile([P, KC, H, D + 1], bf16, tag="v")

        def emit_v_proj(jo):
            for fh in range(2):
                ps = bigps.tile([P, 512], f32, tag="big", name=f"vp_{jo}_{fh}")
                for co in range(KC):
                    nc.tensor.matmul(
                        ps,
                        kinT[:, co, jo * P : (jo + 1) * P],
                        wvT[:, co, fh * 512 : (fh + 1) * 512],
                        start=(co == 0),
                        stop=(co == KC - 1),
                    )
                nc.vector.tensor_scalar_mul(
                    v_s[:, jo, fh * 8 : (fh + 1) * 8, 0:D],
                    ps,
                    mask_s[:, jo : jo + 1],
                )
            nc.vector.tensor_copy(
                v_s[:, jo, :, D : D + 1],
                mask_s[:, jo : jo + 1, None].to_broadcast((P, H, 1)),
            )

        for jo in range(4):
            emit_v_proj(jo)
            zsteps(2)

        for fo in range(KC):
            emit_k_proj(fo, 1)
            zsteps(2)

        for jo in range(4, 8):
            emit_v_proj(jo)
            zsteps(2)

        # ---- Wg + G projection: g = sigmoid(s Wg^T) ----
        wgT = load_wT(wgT_d, "wgT")
        g_s = small_p.tile([P, CS], bf16, tag="g")
        for fh in range(2):
            ps = bigps.tile([P, 512], f32, tag="big", name=f"gp_{fh}")
            for co in range(KC):
                nc.tensor.matmul(
                    ps,
                    sT[:, co, :],
                    wgT[:, co, fh * 512 : (fh + 1) * 512],
                    start=(co == 0),
                    stop=(co == KC - 1),
                )
            nc.scalar.activation(g_s[:, fh * 512 : (fh + 1) * 512], ps, AF.Sigmoid)
            zsteps(2)

        # ---- Wo (consumed only at the tail) ----
        woT = load_wT(woT_d, "woT")

        # ---- attention: 3 passes over j (512, 256, 256) ----
        o_s = small_p.tile([P, CS], bf16, tag="o")
        o_acc = small_p.tile([P, H, D + 1], f32, tag="oacc")

        def emit_attn_h(jq, j0, nj, h, zk):
            fo, pb = h // 2, (h % 2) * D
            qkt = qkps.tile([P, 512], f32, tag="qk", name=f"qk_{jq}_{h}")
            qk = qkt[:, :nj]
            nc.tensor.matmul(
                qk,
                qT[pb : pb + D, fo, :],
                kT[pb : pb + D, fo, j0 : j0 + nj],
                start=True,
                stop=True,
            )
            # add z in place in PSUM, then exp on ACT reading PSUM; the z
            # units emitted in between fill the PE while add+exp run
            nc.vector.tensor_tensor(
                qk,
                qk,
                z_s[:, j0 // 32 : (j0 + nj) // 32, :, h].rearrange("p a b -> p (a b)"),
                ALU.add,
            )
            et = et_p.tile([P, 512], bf16, tag="et", name=f"et_{jq}_{h}")
            nc.scalar.activation(et[:, :nj], qk, AF.Exp)
            zsteps(zk)
            tb = tpsum.tile([P, 1024], bf16, tag="tb", name=f"etb_{jq}_{h}")
            nt = nj // P
            for jl in range(nt):
                nc.tensor.transpose(
                    tb[:, jl * P : (jl + 1) * P], et[:, jl * P : (jl + 1) * P], ident
                )
            etT = et_p.tile([P, 4, P], bf16, tag="etT", name=f"etT_{jq}_{h}")
            nc.vector.tensor_copy(etT[:, :nt, :], tb[:, : nt * P])
            zsteps(zk)
            op = ops.tile([P, 512], f32, tag="op", name=f"op_{jq}_{h}")
            for q in range(nt):
                nc.tensor.matmul(
                    op[:, : D + 1],
                    etT[:, q, :],
                    v_s[:, j0 // P + q, h, :],
                    start=(q == 0),
                    stop=(q == nt - 1),
                )
            if jq == 0:
                nc.vector.tensor_copy(o_acc[:, h, :], op[:, : D + 1])
            else:
                nc.vector.tensor_tensor(
                    o_acc[:, h, :], op[:, : D + 1], o_acc[:, h, :], ALU.add
                )

        # pass 0: j 0..511 (needs z chunks 0-15, kT jh0, v jo0-3)
        z_barrier(64)
        for h in range(H):
            emit_attn_h(0, 0, 512, h, 1)
        # pass 1: j 512..767 (needs z chunks 16-23, kT jh1, v jo4-5)
        z_barrier(96)
        for h in range(H):
            emit_attn_h(1, 512, 256, h, 1)
        # pass 2: j 768..1023
        z_barrier(NUNIT)
        for h in range(H):
            emit_attn_h(2, 768, 256, h, 0)

        # ---- normalize: o = o_num / o_den ----
        for h in range(H):
            rec = r_p.tile([P, 1], f32, tag="r", name=f"rec_{h}")
            nc.vector.reciprocal(rec, o_acc[:, h, D : D + 1])
            nc.vector.tensor_scalar_mul(
                o_s[:, h * D : (h + 1) * D], o_acc[:, h, 0:D], rec
            )

        # ---- gating + output projection ----
        nc.vector.tensor_mul(g_s, g_s, o_s)
        goT = small_p.tile([P, KC, NI], bf16, tag="goT")
        for gh in range(2):
            tb = tpsum.tile([P, 1024], bf16, tag="tb", name=f"gtb_{gh}")
            for fo in range(gh * 4, gh * 4 + 4):
                nc.tensor.transpose(
                    tb[:, (fo % 4) * P : (fo % 4 + 1) * P],
                    g_s[:, fo * P : (fo + 1) * P],
                    ident,
                )
            nc.vector.tensor_copy(goT[:, gh * 4 : (gh + 1) * 4, :], tb[:, : 4 * P])

        for fh in range(2):
            ps = bigps.tile([P, 512], f32, tag="big", name=f"op_ps_{fh}")
            for fo in range(KC):
                nc.tensor.matmul(
                    ps,
                    goT[:, fo, :],
                    woT[:, fo, fh * 512 : (fh + 1) * 512],
                    start=(fo == 0),
                    stop=(fo == KC - 1),
                )
            out_s = outs_p.tile([P, 512], f32, tag="outs", name=f"out_s{fh}")
            nc.vector.tensor_copy(out_s, ps)
            nc.sync.dma_start(out_d[:, fh * 512 : (fh + 1) * 512], out_s)

    nc.compile()
    return nc


def kernel(**inputs):
    global _last_results
    import ml_dtypes

    from concourse.bass_utils import run_bass_kernel_spmd

    bf16 = ml_dtypes.bfloat16
    s = np.asarray(inputs["s"], dtype=np.float32)[0].astype(bf16)
    k_in = np.asarray(inputs["k_in"], dtype=np.float32)[0].astype(bf16)
    mask = np.ascontiguousarray(np.asarray(inputs["mask"], dtype=np.float32)[0])
    bias = np.asarray(inputs["bias"], dtype=np.float32)[0].astype(bf16)
    kinT = np.ascontiguousarray(k_in.T)
    wqT = np.ascontiguousarray(np.asarray(inputs["Wq"], dtype=np.float32).astype(bf16).T)
    wkT = np.ascontiguousarray(np.asarray(inputs["Wk"], dtype=np.float32).astype(bf16).T)
    wvT = np.ascontiguousarray(np.asarray(inputs["Wv"], dtype=np.float32).astype(bf16).T)
    wgT = np.ascontiguousarray(np.asarray(inputs["Wg"], dtype=np.float32).astype(bf16).T)
    woT = np.ascontiguousarray(np.asarray(inputs["Wo"], dtype=np.float32).astype(bf16).T)
    bq = np.ascontiguousarray(np.asarray(inputs["bq"], dtype=np.float32))
    wz = np.asarray(inputs["Wz"], dtype=np.float32).astype(bf16)
    mult = int(np.asarray(inputs.get("multiplicity", 1)))
    assert mult == 1, f"multiplicity={mult} not supported (B=1)"

    nc = _build_program()

    in_maps = []
    for c in range(NCORES):
        sl = slice(c * NI, (c + 1) * NI)
        in_maps.append(
            {
                "sT": np.ascontiguousarray(s[sl].T),
                "biasT": np.ascontiguousarray(bias[sl].transpose(2, 1, 0)),
                "kinT": kinT,
                "mask": mask,
                "wqT": wqT,
                "wkT": wkT,
                "wvT": wvT,
                "wgT": wgT,
                "woT": woT,
                "b_q": bq,
                "w_z": wz,
            }
        )

    try:
        res = run_bass_kernel_spmd(nc, in_maps, core_ids=list(range(NCORES)))
    except Exception:
        # transient device-unrecoverable errors have been observed on a
        # first attempt; one retry has always succeeded
        import time as _time

        _time.sleep(5.0)
        res = run_bass_kernel_spmd(nc, in_maps, core_ids=list(range(NCORES)))
    _last_results = res
    out = np.concatenate([r["out"] for r in res.results], axis=0)
    return out.reshape(B, I, CS).astype(np.float32)


if __name__ == "__main__":
    rng = np.random.default_rng(0)
    ins = {
        "s": rng.standard_normal((B, I, CS), dtype=np.float32),
        "k_in": rng.standard_normal((B, J, CS), dtype=np.float32),
        "mask": np.ones((B, J), np.float32),
        "bias": rng.standard_normal((B, I, J, CZ), dtype=np.float32),
        "Wq": rng.standard_normal((CS, CS), dtype=np.float32) * 0.02,
        "bq": rng.standard_normal((CS,), dtype=np.float32) * 0.02,
        "Wk": rng.standard_normal((CS, CS), dtype=np.float32) * 0.02,
        "Wv": rng.standard_normal((CS, CS), dtype=np.float32) * 0.02,
        "Wg": rng.standard_normal((CS, CS), dtype=np.float32) * 0.02,
        "Wo": rng.standard_normal((CS, CS), dtype=np.float32) * 0.02,
        "Wz": rng.standard_normal((CZ, H), dtype=np.float32) * 0.02,
        "multiplicity": 1,
    }
    out = kernel(**ins)
    print(out.shape, out.dtype)
